# revision 11
# baseline (speedup 1.0000x reference)
"""AdaptiveSpline forward on 8 TRN2 NeuronCores (Bass/Tile).

Math: the reference spline
    out(x) = sum_j coeffs[j] * prod_{i=1..3} clamp((x - t_j)/(t_{j+i} - t_j), 0, 1)
with uniform knots t_k = t0 + k*h is, on each knot interval, an exact cubic
polynomial.  Writing u = (x - t0)/h and s_k = clamp(u - k, 0, 1) it collapses
to the bounded clamped-power basis

    out = A0 + sum_{k=0}^{62} [ gam_k * s_k + bet_k * s_k^2 + alp_k * s_k^3 ]

Device mapping, per knot k (engine-balanced):
    t = clamp(x, X_k, X_{k+1})            [VectorE or GPSIMD dual-op tensor_scalar]
    q = Square(t/h - X_k/h)  (= s^2)      [ScalarE activation, affine folded]
    r = (t - d) * q                       [VectorE scalar_tensor_tensor -> bf16]
        where d = X_k - h*bet/alp, so (alp/h)*r = alp*s^3 + bet*s^2
    psum += (gam/h)*t  + (alp/h)*r        [TensorE scaled-identity matmuls;
                                           fp32r for t, bf16 for r]
Scaled identities are DMA'd in as constant inputs (host-built); TensorE
accumulates everything in PSUM; ScalarE Identity(+A0' bias) evicts.

Sharding: pure data parallel - x split into 8 contiguous shards of 262144,
one per NeuronCore; knots/coeffs fold into immediates + weight tables.
"""

import os
import numpy as np

N_TOTAL = 2_097_152
N_CORES = 8
P = 128
SHARD = N_TOTAL // N_CORES          # 262144
W = SHARD // P                      # 2048 fp32 per partition
BANK = 512                          # psum bank width (fp32)
NB = W // BANK                      # 4 banks

NUM_KNOTS = 64
DEG = 3
NI = NUM_KNOTS - 1                  # 63 intervals / knots in the s-basis

# fraction of clamps sent to GPSIMD (rest on VectorE); tunable
GPS_FRAC = float(os.environ.get("KERNEL_GPS_FRAC", "0.8"))
ALPHA_EPS = 1e-7
DMA_GROUP = 8                       # knots' weights per weight-table DMA

_CACHE: dict = {}


def _tables(knots: np.ndarray, coeffs: np.ndarray):
    """Host-side (float64) per-knot scalars for the clamped-power basis."""
    kd = knots.astype(np.float64)
    cd = coeffs.astype(np.float64)
    K = NUM_KNOTS - 1 - DEG          # 60 basis functions
    h = (kd[-1] - kd[0]) / (NUM_KNOTS - 1)
    assert np.allclose(np.diff(kd), h, rtol=1e-4, atol=1e-6), "knots not uniform"
    t0 = kd[0]

    def c(j):
        return cd[j] if 0 <= j < K else 0.0

    alp = np.zeros(NI)
    bet = np.zeros(NI)
    gam = np.zeros(NI)
    for m in range(NI):
        gam[m] = c(m - 2) / 3.0 + c(m - 1) / 3.0
        bet[m] = c(m - 1) / 6.0
        alp[m] = c(m) / 6.0
    a0 = (2.0 / 3.0) * c(-2) + (1.0 / 6.0) * c(-1)                # A[0] == 0
    return h, t0, a0, alp, bet, gam


def _plan(knots, coeffs):
    h, t0, a0, alp, bet, gam = _tables(knots, coeffs)
    su = 1.0 / h

    plan = []
    fills_r = []      # f32r identity fills (lin-channel rhs=t, sq fallback rhs=q)
    fills_b = []      # bf16 identity fills (cube channel rhs=r)
    const = a0
    for k in range(NI):
        a_, b_, g_ = alp[k], bet[k], gam[k]
        if a_ == 0.0 and b_ == 0.0 and g_ == 0.0:
            continue
        Xk = t0 + k * h
        Xk1 = t0 + (k + 1) * h
        ent = {"k": k, "clamp": (Xk, Xk1), "sq": (su, -Xk * su)}
        cub = abs(a_) > ALPHA_EPS * max(abs(b_), 1.0)
        if g_ != 0.0:
            ent["w_lin"] = len(fills_r)
            fills_r.append(g_ / h)
            const += -g_ * Xk / h
        if cub:
            ent["stt_c"] = Xk - h * b_ / a_
            ent["w_cub"] = len(fills_b)
            fills_b.append(a_ / h)
        elif b_ != 0.0:
            ent["w_sq"] = len(fills_r)
            fills_r.append(b_)
        if not any(key in ent for key in ("w_lin", "w_cub", "w_sq")):
            continue
        plan.append(ent)
    return plan, fills_r, fills_b, const, su, t0, h


def _build(knots: np.ndarray, coeffs: np.ndarray):
    from concourse import bacc, mybir
    import concourse.tile as tile

    plan, fills_r, fills_b, const, su, t0, h = _plan(knots, coeffs)
    nwr, nwb = len(fills_r), len(fills_b)

    nc = bacc.Bacc("TRN2", target_bir_lowering=False, debug=False,
                   num_devices=N_CORES)
    f32 = mybir.dt.float32
    f32r = mybir.dt.float32r
    bf16 = mybir.dt.bfloat16

    x_dram = nc.dram_tensor("x", [SHARD], f32, kind="ExternalInput")
    wr_dram = nc.dram_tensor("wtab_r", [max(nwr, 1) * P * P], f32r,
                             kind="ExternalInput")
    wb_dram = nc.dram_tensor("wtab_b", [max(nwb, 1) * P * P], bf16,
                             kind="ExternalInput")
    out_dram = nc.dram_tensor("out", [SHARD], f32, kind="ExternalOutput")
    x_2d = x_dram.ap().rearrange("(p w) -> p w", p=P)
    out_2d = out_dram.ap().rearrange("(p w) -> p w", p=P)
    # DRAM layout: block w, partition p, col j at w*P*P + p*P + j
    # -> SBUF tile [P, nw*P], block w at [:, w*P:(w+1)*P]
    wr_3d = wr_dram.ap().rearrange("(w p j) -> p w j", p=P, j=P)
    wb_3d = wb_dram.ap().rearrange("(w p j) -> p w j", p=P, j=P)

    # ScalarE activation biases must be [P,1] APs
    bias_vals: list[float] = []

    def bias_idx(v: float) -> int:
        v = float(np.float32(v))
        if v not in bias_vals:
            bias_vals.append(v)
        return bias_vals.index(v)

    for ent in plan:
        ent["sq_bias_i"] = bias_idx(ent["sq"][1])
    const_i = bias_idx(const)

    # spread GPSIMD clamps evenly through the knot list
    n_gps = int(round(GPS_FRAC * len(plan)))
    acc_g = 0.0
    for ent in plan:
        acc_g += n_gps / len(plan)
        if acc_g >= 1.0 - 1e-9:
            acc_g -= 1.0
            ent["clamp_eng"] = "gps"
        else:
            ent["clamp_eng"] = "dve"

    with tile.TileContext(nc) as tc:
        with (
            tc.tile_pool(name="const", bufs=1) as cpool,
            tc.tile_pool(name="work", bufs=3) as work,
            tc.tile_pool(name="outp", bufs=1) as outp,
            tc.tile_pool(name="psum", bufs=1, space="PSUM") as psum,
        ):
            biases = cpool.tile([P, max(len(bias_vals), 1)], f32, tag="biases")
            for bi, bv in enumerate(bias_vals):
                nc.gpsimd.memset(biases[:, bi:bi + 1], bv)

            identr = cpool.tile([P, max(nwr, 1) * P], f32r, tag="identr")
            identb = cpool.tile([P, max(nwb, 1) * P], bf16, tag="identb")
            for lo in range(0, nwr, DMA_GROUP):
                hi = min(lo + DMA_GROUP, nwr)
                nc.sync.dma_start(
                    out=identr[:, lo * P:hi * P].rearrange(
                        "p (w j) -> p w j", j=P),
                    in_=wr_3d[:, lo:hi, :])
            for lo in range(0, nwb, DMA_GROUP):
                hi = min(lo + DMA_GROUP, nwb)
                nc.sync.dma_start(
                    out=identb[:, lo * P:hi * P].rearrange(
                        "p (w j) -> p w j", j=P),
                    in_=wb_3d[:, lo:hi, :])

            x_tile = cpool.tile([P, W], f32, tag="x")
            nc.sync.dma_start(out=x_tile[:], in_=x_2d)

            acc = [psum.tile([P, BANK], f32, tag=f"ps{b}", name=f"ps{b}")
                   for b in range(NB)]
            started = [False] * NB
            n_mm = sum(1 for e in plan for key in ("w_lin", "w_cub", "w_sq")
                       if key in e)
            mm_done = [0] * NB

            def mm(b, ident, wi, rhs):
                first = not started[b]
                started[b] = True
                mm_done[b] += 1
                nc.tensor.matmul(
                    acc[b][:],
                    ident[:, wi * P:(wi + 1) * P],
                    rhs[:, b * BANK:(b + 1) * BANK],
                    start=first,
                    stop=(mm_done[b] == n_mm),
                )

            for ent in plan:
                Xk, Xk1 = ent["clamp"]
                tt = work.tile([P, W], f32r, tag="t", name=f"t{ent['k']}")
                eng = nc.gpsimd if ent["clamp_eng"] == "gps" else nc.vector
                eng.tensor_scalar(tt[:], x_tile[:], float(Xk), float(Xk1),
                                  mybir.AluOpType.max, mybir.AluOpType.min)
                if "w_lin" in ent:
                    for b in range(NB):
                        mm(b, identr, ent["w_lin"], tt)
                if "w_cub" in ent or "w_sq" in ent:
                    qt = work.tile([P, W], f32r, tag="q", name=f"q{ent['k']}")
                    sc, _bi = ent["sq"]
                    nc.scalar.activation(
                        qt[:], tt[:], mybir.ActivationFunctionType.Square,
                        bias=biases[:, ent["sq_bias_i"]:ent["sq_bias_i"] + 1],
                        scale=float(sc))
                    if "w_cub" in ent:
                        rt = work.tile([P, W], bf16, tag="r", name=f"r{ent['k']}")
                        nc.vector.scalar_tensor_tensor(
                            rt[:], tt[:], float(ent["stt_c"]), qt[:],
                            mybir.AluOpType.subtract, mybir.AluOpType.mult)
                        for b in range(NB):
                            mm(b, identb, ent["w_cub"], rt)
                    else:
                        for b in range(NB):
                            mm(b, identr, ent["w_sq"], qt)

            out_tile = outp.tile([P, W], f32, tag="o")
            for b in range(NB):
                nc.scalar.activation(
                    out_tile[:, b * BANK:(b + 1) * BANK], acc[b][:],
                    mybir.ActivationFunctionType.Identity,
                    bias=biases[:, const_i:const_i + 1], scale=1.0)
            nc.sync.dma_start(out=out_2d, in_=out_tile[:])

    nc.compile()

    # host-side constant weight tables (scaled identity blocks)
    eye = np.eye(P, dtype=np.float64)
    wtab_r = (np.stack([eye * v for v in fills_r]).astype(np.float32).reshape(-1)
              if fills_r else np.zeros(P * P, np.float32))
    try:
        import ml_dtypes
        bft = ml_dtypes.bfloat16
    except ImportError:
        import jax.numpy as jnp
        bft = jnp.bfloat16
    wtab_b = (np.stack([eye * v for v in fills_b]).astype(bft).reshape(-1)
              if fills_b else np.zeros(P * P, bft))
    return nc, wtab_r, wtab_b


def _get_nc(knots: np.ndarray, coeffs: np.ndarray):
    key = (knots.astype(np.float32).tobytes(),
           coeffs.astype(np.float32).tobytes(), GPS_FRAC)
    if key not in _CACHE:
        _CACHE[key] = _build(knots, coeffs)
    return _CACHE[key]


LAST_RESULT = None


def _ensure_trace_hook() -> bool:
    """The image's antenv lacks axon_hooks; shim it so trace=True works."""
    try:
        from antenv.axon_hooks import get_axon_ntff_profile_hook  # noqa: F401
        return True
    except ImportError:
        pass
    try:
        import sys
        import types
        mod = types.ModuleType("antenv.axon_hooks")
        mod._hook = None

        def set_axon_ntff_profile_hook(h):
            mod._hook = h

        def get_axon_ntff_profile_hook():
            return mod._hook

        mod.set_axon_ntff_profile_hook = set_axon_ntff_profile_hook
        mod.get_axon_ntff_profile_hook = get_axon_ntff_profile_hook
        sys.modules["antenv.axon_hooks"] = mod
        import antenv
        antenv.axon_hooks = mod
        from trn_agent_boot.trn_boot import _ntff_profile_via_ctypes
        hook = _ntff_profile_via_ctypes("/opt/axon/libaxon_pjrt.so")
        mod._hook = hook
        return hook is not None
    except Exception:
        return False


def kernel(x: np.ndarray, knots: np.ndarray, coeffs: np.ndarray) -> np.ndarray:
    global LAST_RESULT
    from concourse.bass_utils import run_bass_kernel_spmd

    x = np.ascontiguousarray(np.asarray(x, dtype=np.float32))
    assert x.shape == (N_TOTAL,)
    nc, wtab_r, wtab_b = _get_nc(np.asarray(knots), np.asarray(coeffs))

    shards = x.reshape(N_CORES, SHARD)
    in_maps = [{"x": np.ascontiguousarray(shards[i]),
                "wtab_r": wtab_r, "wtab_b": wtab_b}
               for i in range(N_CORES)]
    trace = bool(int(os.environ.get("KERNEL_TRACE", "0")))
    if trace:
        trace = _ensure_trace_hook()
    res = run_bass_kernel_spmd(
        nc, in_maps, core_ids=list(range(N_CORES)), trace=trace)
    LAST_RESULT = res
    out = np.concatenate([res.results[i]["out"].reshape(-1)
                          for i in range(N_CORES)])
    return out.astype(np.float32, copy=False)


# revision 13
# speedup vs baseline: 6.7308x; 6.7308x over previous
"""AdaptiveSpline forward on 8 TRN2 NeuronCores (Bass/Tile).

Math: the reference spline
    out(x) = sum_j coeffs[j] * prod_{i=1..3} clamp((x - t_j)/(t_{j+i} - t_j), 0, 1)
with uniform knots t_k = t0 + k*h is, on each knot interval, an exact cubic
polynomial.  Writing u = (x - t0)/h and s_k = clamp(u - k, 0, 1) it collapses
to the bounded clamped-power basis

    out = A0 + sum_{k=0}^{62} [ gam_k * s_k + bet_k * s_k^2 + alp_k * s_k^3 ]

Device mapping, per knot k (engine-balanced):
    t = clamp(x, X_k, X_{k+1})            [VectorE or GPSIMD dual-op tensor_scalar]
    q = Square(t/h - X_k/h)  (= s^2)      [ScalarE activation, affine folded]
    r = (t - d) * q                       [VectorE scalar_tensor_tensor -> bf16]
        where d = X_k - h*bet/alp, so (alp/h)*r = alp*s^3 + bet*s^2
    psum += (gam/h)*t  + (alp/h)*r        [TensorE scaled-identity matmuls;
                                           fp32r for t, bf16 for r]
Scaled identities are DMA'd in as constant inputs (host-built); TensorE
accumulates everything in PSUM; ScalarE Identity(+A0' bias) evicts.

Sharding: pure data parallel - x split into 8 contiguous shards of 262144,
one per NeuronCore; knots/coeffs fold into immediates + weight tables.
"""

import os
import numpy as np

N_TOTAL = 2_097_152
N_CORES = 8
P = 128
SHARD = N_TOTAL // N_CORES          # 262144
W = SHARD // P                      # 2048 fp32 per partition
BANK = 512                          # psum bank width (fp32)
NB = W // BANK                      # 4 banks

NUM_KNOTS = 64
DEG = 3
NI = NUM_KNOTS - 1                  # 63 intervals / knots in the s-basis

# fraction of knots whose clamp runs as two ScalarE Relus (recipe A);
# the rest clamp on VectorE (recipe C).  Tuned so ScE and DVE finish together.
LAMBDA_A = float(os.environ.get("KERNEL_LAMBDA_A", "0.30"))
ALPHA_EPS = 1e-7
DMA_GROUP = 8                       # knots' weights per weight-table DMA

_CACHE: dict = {}


def _tables(knots: np.ndarray, coeffs: np.ndarray):
    """Host-side (float64) per-knot scalars for the clamped-power basis."""
    kd = knots.astype(np.float64)
    cd = coeffs.astype(np.float64)
    K = NUM_KNOTS - 1 - DEG          # 60 basis functions
    h = (kd[-1] - kd[0]) / (NUM_KNOTS - 1)
    assert np.allclose(np.diff(kd), h, rtol=1e-4, atol=1e-6), "knots not uniform"
    t0 = kd[0]

    def c(j):
        return cd[j] if 0 <= j < K else 0.0

    alp = np.zeros(NI)
    bet = np.zeros(NI)
    gam = np.zeros(NI)
    for m in range(NI):
        gam[m] = c(m - 2) / 3.0 + c(m - 1) / 3.0
        bet[m] = c(m - 1) / 6.0
        alp[m] = c(m) / 6.0
    a0 = (2.0 / 3.0) * c(-2) + (1.0 / 6.0) * c(-1)                # A[0] == 0
    return h, t0, a0, alp, bet, gam


def _plan(knots, coeffs):
    h, t0, a0, alp, bet, gam = _tables(knots, coeffs)
    su = 1.0 / h

    plan = []
    fills_r = []      # f32r identity fills (lin-channel rhs=t, sq fallback rhs=q)
    fills_b = []      # bf16 identity fills (cube channel rhs=r)
    const = a0
    for k in range(NI):
        a_, b_, g_ = alp[k], bet[k], gam[k]
        if a_ == 0.0 and b_ == 0.0 and g_ == 0.0:
            continue
        Xk = t0 + k * h
        Xk1 = t0 + (k + 1) * h
        ent = {"k": k}
        cub = abs(a_) > ALPHA_EPS * max(abs(b_), 1.0)
        use_a = (len([e for e in plan if e["recipe"] == "A"]) + 1) \
            / (len(plan) + 1) <= LAMBDA_A
        ent["recipe"] = "A" if use_a else "C"
        if use_a:
            # w = Relu(su*x + bu - k); v = Relu(1 - w) = 1 - s
            ent["relu_bias"] = -t0 / h - k
            ent["sq"] = (-1.0, 1.0)             # Square(1 - v) = s^2
            if g_ != 0.0:
                ent["w_lin"] = len(fills_r)
                fills_r.append(-g_)
                const += g_
            if cub:
                ent["stt_c"] = (a_ + b_) / a_   # r = (v - c)*q = -(s^3 + (b/a) s^2)
                ent["w_cub"] = len(fills_r)
                fills_r.append(-a_)
            elif b_ != 0.0:
                ent["w_sq"] = len(fills_r)
                fills_r.append(b_)
        else:
            ent["clamp"] = (Xk, Xk1)
            ent["sq"] = (su, -Xk * su)          # Square(su*t - su*Xk) = s^2
            if g_ != 0.0:
                ent["w_lin"] = len(fills_r)
                fills_r.append(g_ / h)
                const += -g_ * Xk / h
            if cub:
                ent["stt_c"] = Xk - h * b_ / a_
                ent["w_cub"] = len(fills_r)
                fills_r.append(a_ / h)
            elif b_ != 0.0:
                ent["w_sq"] = len(fills_r)
                fills_r.append(b_)
        if not any(key in ent for key in ("w_lin", "w_cub", "w_sq")):
            continue
        plan.append(ent)
    return plan, fills_r, fills_b, const, su, t0, h


def _build(knots: np.ndarray, coeffs: np.ndarray):
    from concourse import bacc, mybir
    import concourse.tile as tile

    plan, fills_r, fills_b, const, su, t0, h = _plan(knots, coeffs)
    nwr, nwb = len(fills_r), len(fills_b)

    nc = bacc.Bacc("TRN2", target_bir_lowering=False, debug=False,
                   num_devices=N_CORES)
    f32 = mybir.dt.float32
    f32r = mybir.dt.float32r
    bf16 = mybir.dt.bfloat16

    x_dram = nc.dram_tensor("x", [SHARD], f32, kind="ExternalInput")
    wr_dram = nc.dram_tensor("wtab_r", [max(nwr, 1) * P * P], f32r,
                             kind="ExternalInput")
    out_dram = nc.dram_tensor("out", [SHARD], f32, kind="ExternalOutput")
    x_2d = x_dram.ap().rearrange("(p w) -> p w", p=P)
    out_2d = out_dram.ap().rearrange("(p w) -> p w", p=P)
    # DRAM layout: block w, partition p, col j at w*P*P + p*P + j
    # -> SBUF tile [P, nw*P], block w at [:, w*P:(w+1)*P]
    wr_3d = wr_dram.ap().rearrange("(w p j) -> p w j", p=P, j=P)

    # ScalarE activation biases must be [P,1] APs
    bias_vals: list[float] = []

    def bias_idx(v: float) -> int:
        v = float(np.float32(v))
        if v not in bias_vals:
            bias_vals.append(v)
        return bias_vals.index(v)

    for ent in plan:
        ent["sq_bias_i"] = bias_idx(ent["sq"][1])
        if ent["recipe"] == "A":
            ent["relu_bias_i"] = bias_idx(ent["relu_bias"])
            ent["one_i"] = bias_idx(1.0)
    const_i = bias_idx(const)

    with tile.TileContext(nc) as tc:
        with (
            tc.tile_pool(name="const", bufs=1) as cpool,
            tc.tile_pool(name="work", bufs=3) as work,
            tc.tile_pool(name="outp", bufs=1) as outp,
            tc.tile_pool(name="psum", bufs=1, space="PSUM") as psum,
        ):
            biases = cpool.tile([P, max(len(bias_vals), 1)], f32, tag="biases")
            for bi, bv in enumerate(bias_vals):
                nc.gpsimd.memset(biases[:, bi:bi + 1], bv)

            identr = cpool.tile([P, max(nwr, 1) * P], f32r, tag="identr")
            for lo in range(0, nwr, DMA_GROUP):
                hi = min(lo + DMA_GROUP, nwr)
                nc.sync.dma_start(
                    out=identr[:, lo * P:hi * P].rearrange(
                        "p (w j) -> p w j", j=P),
                    in_=wr_3d[:, lo:hi, :])

            x_tile = cpool.tile([P, W], f32, tag="x")
            nc.sync.dma_start(out=x_tile[:], in_=x_2d)

            acc = [psum.tile([P, BANK], f32, tag=f"ps{b}", name=f"ps{b}")
                   for b in range(NB)]
            started = [False] * NB
            n_mm = sum(1 for e in plan for key in ("w_lin", "w_cub", "w_sq")
                       if key in e)
            mm_done = [0] * NB

            def mm(b, ident, wi, rhs):
                first = not started[b]
                started[b] = True
                mm_done[b] += 1
                nc.tensor.matmul(
                    acc[b][:],
                    ident[:, wi * P:(wi + 1) * P],
                    rhs[:, b * BANK:(b + 1) * BANK],
                    start=first,
                    stop=(mm_done[b] == n_mm),
                )

            su = 1.0 / h
            for ent in plan:
                k = ent["k"]
                if ent["recipe"] == "A":
                    wt = work.tile([P, W], f32r, tag="w", name=f"w{k}")
                    nc.scalar.activation(
                        wt[:], x_tile[:], mybir.ActivationFunctionType.Relu,
                        bias=biases[:, ent["relu_bias_i"]:ent["relu_bias_i"] + 1],
                        scale=float(su))
                    lin = work.tile([P, W], f32r, tag="v", name=f"v{k}")
                    nc.scalar.activation(
                        lin[:], wt[:], mybir.ActivationFunctionType.Relu,
                        bias=biases[:, ent["one_i"]:ent["one_i"] + 1],
                        scale=-1.0)
                else:
                    Xk, Xk1 = ent["clamp"]
                    lin = work.tile([P, W], f32r, tag="t", name=f"t{k}")
                    nc.vector.tensor_scalar(
                        lin[:], x_tile[:], float(Xk), float(Xk1),
                        mybir.AluOpType.max, mybir.AluOpType.min)
                if "w_lin" in ent:
                    for b in range(NB):
                        mm(b, identr, ent["w_lin"], lin)
                if "w_cub" in ent or "w_sq" in ent:
                    qt = work.tile([P, W], f32r, tag="q", name=f"q{k}")
                    sc, _bi = ent["sq"]
                    nc.scalar.activation(
                        qt[:], lin[:], mybir.ActivationFunctionType.Square,
                        bias=biases[:, ent["sq_bias_i"]:ent["sq_bias_i"] + 1],
                        scale=float(sc))
                    if "w_cub" in ent:
                        rt = work.tile([P, W], f32r, tag="r", name=f"r{k}")
                        nc.vector.scalar_tensor_tensor(
                            rt[:], lin[:], float(ent["stt_c"]), qt[:],
                            mybir.AluOpType.subtract, mybir.AluOpType.mult)
                        for b in range(NB):
                            mm(b, identr, ent["w_cub"], rt)
                    else:
                        for b in range(NB):
                            mm(b, identr, ent["w_sq"], qt)

            out_tile = outp.tile([P, W], f32, tag="o")
            for b in range(NB):
                nc.scalar.activation(
                    out_tile[:, b * BANK:(b + 1) * BANK], acc[b][:],
                    mybir.ActivationFunctionType.Identity,
                    bias=biases[:, const_i:const_i + 1], scale=1.0)
            nc.sync.dma_start(out=out_2d, in_=out_tile[:])

    nc.compile()

    # host-side constant weight tables (scaled identity blocks)
    eye = np.eye(P, dtype=np.float64)
    wtab_r = (np.stack([eye * v for v in fills_r]).astype(np.float32).reshape(-1)
              if fills_r else np.zeros(P * P, np.float32))
    return nc, wtab_r


def _get_nc(knots: np.ndarray, coeffs: np.ndarray):
    key = (knots.astype(np.float32).tobytes(),
           coeffs.astype(np.float32).tobytes(), LAMBDA_A)
    if key not in _CACHE:
        _CACHE[key] = _build(knots, coeffs)
    return _CACHE[key]


LAST_RESULT = None


def _ensure_trace_hook() -> bool:
    """The image's antenv lacks axon_hooks; shim it so trace=True works."""
    try:
        from antenv.axon_hooks import get_axon_ntff_profile_hook  # noqa: F401
        return True
    except ImportError:
        pass
    try:
        import sys
        import types
        mod = types.ModuleType("antenv.axon_hooks")
        mod._hook = None

        def set_axon_ntff_profile_hook(h):
            mod._hook = h

        def get_axon_ntff_profile_hook():
            return mod._hook

        mod.set_axon_ntff_profile_hook = set_axon_ntff_profile_hook
        mod.get_axon_ntff_profile_hook = get_axon_ntff_profile_hook
        sys.modules["antenv.axon_hooks"] = mod
        import antenv
        antenv.axon_hooks = mod
        from trn_agent_boot.trn_boot import _ntff_profile_via_ctypes
        hook = _ntff_profile_via_ctypes("/opt/axon/libaxon_pjrt.so")
        mod._hook = hook
        return hook is not None
    except Exception:
        return False


def kernel(x: np.ndarray, knots: np.ndarray, coeffs: np.ndarray) -> np.ndarray:
    global LAST_RESULT
    from concourse.bass_utils import run_bass_kernel_spmd

    x = np.ascontiguousarray(np.asarray(x, dtype=np.float32))
    assert x.shape == (N_TOTAL,)
    nc, wtab_r = _get_nc(np.asarray(knots), np.asarray(coeffs))

    shards = x.reshape(N_CORES, SHARD)
    in_maps = [{"x": np.ascontiguousarray(shards[i]), "wtab_r": wtab_r}
               for i in range(N_CORES)]
    trace = bool(int(os.environ.get("KERNEL_TRACE", "0")))
    if trace:
        trace = _ensure_trace_hook()
    res = run_bass_kernel_spmd(
        nc, in_maps, core_ids=list(range(N_CORES)), trace=trace)
    LAST_RESULT = res
    out = np.concatenate([res.results[i]["out"].reshape(-1)
                          for i in range(N_CORES)])
    return out.astype(np.float32, copy=False)


# revision 15
# speedup vs baseline: 8.7252x; 1.2963x over previous
"""AdaptiveSpline forward on 8 TRN2 NeuronCores (Bass/Tile).

Math: the reference spline
    out(x) = sum_j coeffs[j] * prod_{i=1..3} clamp((x - t_j)/(t_{j+i} - t_j), 0, 1)
with uniform knots t_k = t0 + k*h is, on each knot interval, an exact cubic
polynomial.  Writing u = (x - t0)/h and s_k = clamp(u - k, 0, 1) it collapses
to the bounded clamped-power basis

    out = A0 + sum_{k=0}^{62} [ gam_k * s_k + bet_k * s_k^2 + alp_k * s_k^3 ]

Device mapping, per knot k (engine-balanced):
    t = clamp(x, X_k, X_{k+1})            [VectorE or GPSIMD dual-op tensor_scalar]
    q = Square(t/h - X_k/h)  (= s^2)      [ScalarE activation, affine folded]
    r = (t - d) * q                       [VectorE scalar_tensor_tensor -> bf16]
        where d = X_k - h*bet/alp, so (alp/h)*r = alp*s^3 + bet*s^2
    psum += (gam/h)*t  + (alp/h)*r        [TensorE scaled-identity matmuls;
                                           fp32r for t, bf16 for r]
Scaled identities are DMA'd in as constant inputs (host-built); TensorE
accumulates everything in PSUM; ScalarE Identity(+A0' bias) evicts.

Sharding: pure data parallel - x split into 8 contiguous shards of 262144,
one per NeuronCore; knots/coeffs fold into immediates + weight tables.
"""

import os
import numpy as np

N_TOTAL = 2_097_152
N_CORES = 8
P = 128
SHARD = N_TOTAL // N_CORES          # 262144
W = SHARD // P                      # 2048 fp32 per partition
BANK = 512                          # psum bank width (fp32)
NB = W // BANK                      # 4 banks

NUM_KNOTS = 64
DEG = 3
NI = NUM_KNOTS - 1                  # 63 intervals / knots in the s-basis

# fraction of knots whose clamp runs as two ScalarE Relus (recipe A);
# the rest clamp on VectorE (recipe C).  Tuned so ScE and DVE finish together.
LAMBDA_A = float(os.environ.get("KERNEL_LAMBDA_A", "0.30"))
ALPHA_EPS = 1e-7
DMA_GROUP = 8                       # knots' weights per weight-table DMA

_CACHE: dict = {}


def _tables(knots: np.ndarray, coeffs: np.ndarray):
    """Host-side (float64) per-knot scalars for the clamped-power basis."""
    kd = knots.astype(np.float64)
    cd = coeffs.astype(np.float64)
    K = NUM_KNOTS - 1 - DEG          # 60 basis functions
    h = (kd[-1] - kd[0]) / (NUM_KNOTS - 1)
    assert np.allclose(np.diff(kd), h, rtol=1e-4, atol=1e-6), "knots not uniform"
    t0 = kd[0]

    def c(j):
        return cd[j] if 0 <= j < K else 0.0

    alp = np.zeros(NI)
    bet = np.zeros(NI)
    gam = np.zeros(NI)
    for m in range(NI):
        gam[m] = c(m - 2) / 3.0 + c(m - 1) / 3.0
        bet[m] = c(m - 1) / 6.0
        alp[m] = c(m) / 6.0
    a0 = (2.0 / 3.0) * c(-2) + (1.0 / 6.0) * c(-1)                # A[0] == 0
    return h, t0, a0, alp, bet, gam


def _plan(knots, coeffs):
    h, t0, a0, alp, bet, gam = _tables(knots, coeffs)
    su = 1.0 / h

    plan = []
    fills_r = []      # f32r identity fills (lin-channel rhs=t, sq fallback rhs=q)
    fills_b = []      # bf16 identity fills (cube channel rhs=r)
    const = a0
    for k in range(NI):
        a_, b_, g_ = alp[k], bet[k], gam[k]
        if a_ == 0.0 and b_ == 0.0 and g_ == 0.0:
            continue
        Xk = t0 + k * h
        Xk1 = t0 + (k + 1) * h
        ent = {"k": k}
        cub = abs(a_) > ALPHA_EPS * max(abs(b_), 1.0)
        use_a = (len([e for e in plan if e["recipe"] == "A"]) + 1) \
            / (len(plan) + 1) <= LAMBDA_A
        ent["recipe"] = "A" if use_a else "C"
        if use_a:
            # w = Relu(su*x + bu - k); v = Relu(1 - w) = 1 - s
            ent["relu_bias"] = -t0 / h - k
            ent["sq"] = (-1.0, 1.0)             # Square(1 - v) = s^2
            if g_ != 0.0:
                ent["w_lin"] = len(fills_r)
                fills_r.append(-g_)
                const += g_
            if cub:
                ent["stt_c"] = (a_ + b_) / a_   # r = (v - c)*q = -(s^3 + (b/a) s^2)
                ent["w_cub"] = len(fills_r)
                fills_r.append(-a_)
            elif b_ != 0.0:
                ent["w_sq"] = len(fills_r)
                fills_r.append(b_)
        else:
            ent["clamp"] = (Xk, Xk1)
            ent["sq"] = (su, -Xk * su)          # Square(su*t - su*Xk) = s^2
            if g_ != 0.0:
                ent["w_lin"] = len(fills_r)
                fills_r.append(g_ / h)
                const += -g_ * Xk / h
            if cub:
                ent["stt_c"] = Xk - h * b_ / a_
                ent["w_cub"] = len(fills_r)
                fills_r.append(a_ / h)
            elif b_ != 0.0:
                ent["w_sq"] = len(fills_r)
                fills_r.append(b_)
        if not any(key in ent for key in ("w_lin", "w_cub", "w_sq")):
            continue
        plan.append(ent)
    return plan, fills_r, fills_b, const, su, t0, h


def _build(knots: np.ndarray, coeffs: np.ndarray):
    from concourse import bacc, mybir
    import concourse.tile as tile

    plan, fills_r, fills_b, const, su, t0, h = _plan(knots, coeffs)
    nwr, nwb = len(fills_r), len(fills_b)

    nc = bacc.Bacc("TRN2", target_bir_lowering=False, debug=False,
                   num_devices=N_CORES)
    f32 = mybir.dt.float32
    f32r = mybir.dt.float32r
    bf16 = mybir.dt.bfloat16

    x_dram = nc.dram_tensor("x", [SHARD], f32, kind="ExternalInput")
    out_dram = nc.dram_tensor("out", [SHARD], f32, kind="ExternalOutput")
    x_2d = x_dram.ap().rearrange("(p w) -> p w", p=P)
    out_2d = out_dram.ap().rearrange("(p w) -> p w", p=P)

    # ScalarE activation biases must be [P,1] APs
    bias_vals: list[float] = []

    def bias_idx(v: float) -> int:
        v = float(np.float32(v))
        if v not in bias_vals:
            bias_vals.append(v)
        return bias_vals.index(v)

    for ent in plan:
        ent["sq_bias_i"] = bias_idx(ent["sq"][1])
        if ent["recipe"] == "A":
            ent["relu_bias_i"] = bias_idx(ent["relu_bias"])
            ent["one_i"] = bias_idx(1.0)
    const_i = bias_idx(const)

    with tile.TileContext(nc) as tc:
        with (
            tc.tile_pool(name="const", bufs=1) as cpool,
            tc.tile_pool(name="work", bufs=3) as work,
            tc.tile_pool(name="outp", bufs=1) as outp,
            tc.tile_pool(name="psum", bufs=1, space="PSUM") as psum,
        ):
            biases = cpool.tile([P, max(len(bias_vals), 1)], f32, tag="biases")
            for bi, bv in enumerate(bias_vals):
                nc.gpsimd.memset(biases[:, bi:bi + 1], bv)

            identr = cpool.tile([P, max(nwr, 1) * P], f32r, tag="identr")
            zcol = cpool.tile([P, 1], f32, tag="zcol")
            nc.gpsimd.memset(zcol[:], 0.0)
            zbc = zcol[:].broadcast_to([P, P])
            for wi, val in enumerate(fills_r):
                nc.gpsimd.affine_select(
                    out=identr[:, wi * P:(wi + 1) * P],
                    in_=zbc,
                    compare_op=mybir.AluOpType.not_equal,
                    fill=float(val),
                    base=0,
                    pattern=[[-1, P]],
                    channel_multiplier=1,
                )

            x_tile = cpool.tile([P, W], f32, tag="x")
            nc.sync.dma_start(out=x_tile[:], in_=x_2d)

            acc = [psum.tile([P, BANK], f32, tag=f"ps{b}", name=f"ps{b}")
                   for b in range(NB)]
            started = [False] * NB
            n_mm = sum(1 for e in plan for key in ("w_lin", "w_cub", "w_sq")
                       if key in e)
            mm_done = [0] * NB

            def mm(b, ident, wi, rhs):
                first = not started[b]
                started[b] = True
                mm_done[b] += 1
                nc.tensor.matmul(
                    acc[b][:],
                    ident[:, wi * P:(wi + 1) * P],
                    rhs[:, b * BANK:(b + 1) * BANK],
                    start=first,
                    stop=(mm_done[b] == n_mm),
                )

            su = 1.0 / h
            for ent in plan:
                k = ent["k"]
                if ent["recipe"] == "A":
                    wt = work.tile([P, W], f32r, tag="w", name=f"w{k}")
                    nc.scalar.activation(
                        wt[:], x_tile[:], mybir.ActivationFunctionType.Relu,
                        bias=biases[:, ent["relu_bias_i"]:ent["relu_bias_i"] + 1],
                        scale=float(su))
                    lin = work.tile([P, W], f32r, tag="v", name=f"v{k}")
                    nc.scalar.activation(
                        lin[:], wt[:], mybir.ActivationFunctionType.Relu,
                        bias=biases[:, ent["one_i"]:ent["one_i"] + 1],
                        scale=-1.0)
                else:
                    Xk, Xk1 = ent["clamp"]
                    lin = work.tile([P, W], f32r, tag="t", name=f"t{k}")
                    nc.vector.tensor_scalar(
                        lin[:], x_tile[:], float(Xk), float(Xk1),
                        mybir.AluOpType.max, mybir.AluOpType.min)
                if "w_lin" in ent:
                    for b in range(NB):
                        mm(b, identr, ent["w_lin"], lin)
                if "w_cub" in ent or "w_sq" in ent:
                    qt = work.tile([P, W], f32r, tag="q", name=f"q{k}")
                    sc, _bi = ent["sq"]
                    nc.scalar.activation(
                        qt[:], lin[:], mybir.ActivationFunctionType.Square,
                        bias=biases[:, ent["sq_bias_i"]:ent["sq_bias_i"] + 1],
                        scale=float(sc))
                    if "w_cub" in ent:
                        rt = work.tile([P, W], f32r, tag="r", name=f"r{k}")
                        nc.vector.scalar_tensor_tensor(
                            rt[:], lin[:], float(ent["stt_c"]), qt[:],
                            mybir.AluOpType.subtract, mybir.AluOpType.mult)
                        for b in range(NB):
                            mm(b, identr, ent["w_cub"], rt)
                    else:
                        for b in range(NB):
                            mm(b, identr, ent["w_sq"], qt)

            out_tile = outp.tile([P, W], f32, tag="o")
            for b in range(NB):
                nc.scalar.activation(
                    out_tile[:, b * BANK:(b + 1) * BANK], acc[b][:],
                    mybir.ActivationFunctionType.Identity,
                    bias=biases[:, const_i:const_i + 1], scale=1.0)
            nc.sync.dma_start(out=out_2d, in_=out_tile[:])

    nc.compile()

    return nc


def _get_nc(knots: np.ndarray, coeffs: np.ndarray):
    key = (knots.astype(np.float32).tobytes(),
           coeffs.astype(np.float32).tobytes(), LAMBDA_A)
    if key not in _CACHE:
        _CACHE[key] = _build(knots, coeffs)
    return _CACHE[key]


LAST_RESULT = None


def _ensure_trace_hook() -> bool:
    """The image's antenv lacks axon_hooks; shim it so trace=True works."""
    try:
        from antenv.axon_hooks import get_axon_ntff_profile_hook  # noqa: F401
        return True
    except ImportError:
        pass
    try:
        import sys
        import types
        mod = types.ModuleType("antenv.axon_hooks")
        mod._hook = None

        def set_axon_ntff_profile_hook(h):
            mod._hook = h

        def get_axon_ntff_profile_hook():
            return mod._hook

        mod.set_axon_ntff_profile_hook = set_axon_ntff_profile_hook
        mod.get_axon_ntff_profile_hook = get_axon_ntff_profile_hook
        sys.modules["antenv.axon_hooks"] = mod
        import antenv
        antenv.axon_hooks = mod
        from trn_agent_boot.trn_boot import _ntff_profile_via_ctypes
        hook = _ntff_profile_via_ctypes("/opt/axon/libaxon_pjrt.so")
        mod._hook = hook
        return hook is not None
    except Exception:
        return False


def kernel(x: np.ndarray, knots: np.ndarray, coeffs: np.ndarray) -> np.ndarray:
    global LAST_RESULT
    from concourse.bass_utils import run_bass_kernel_spmd

    x = np.ascontiguousarray(np.asarray(x, dtype=np.float32))
    assert x.shape == (N_TOTAL,)
    nc = _get_nc(np.asarray(knots), np.asarray(coeffs))

    shards = x.reshape(N_CORES, SHARD)
    in_maps = [{"x": np.ascontiguousarray(shards[i])} for i in range(N_CORES)]
    trace = bool(int(os.environ.get("KERNEL_TRACE", "0")))
    if trace:
        trace = _ensure_trace_hook()
    res = run_bass_kernel_spmd(
        nc, in_maps, core_ids=list(range(N_CORES)), trace=trace)
    LAST_RESULT = res
    out = np.concatenate([res.results[i]["out"].reshape(-1)
                          for i in range(N_CORES)])
    return out.astype(np.float32, copy=False)


# revision 16
# speedup vs baseline: 9.1222x; 1.0455x over previous
"""AdaptiveSpline forward on 8 TRN2 NeuronCores (Bass/Tile).

Math: the reference spline
    out(x) = sum_j coeffs[j] * prod_{i=1..3} clamp((x - t_j)/(t_{j+i} - t_j), 0, 1)
with uniform knots t_k = t0 + k*h is, on each knot interval, an exact cubic
polynomial.  Writing u = (x - t0)/h and s_k = clamp(u - k, 0, 1) it collapses
to the bounded clamped-power basis

    out = A0 + sum_{k=0}^{62} [ gam_k*s_k + bet_k*s_k^2 + alp_k*s_k^3 ]

Per knot the device evaluates the three channels with one of three
engine-balanced recipes (s^3 and s^2 fold into one product via a root shift;
all matmul weights fold into ScalarE's free affine so TensorE identities are
unscaled +/-1 except the linear channel):

  C  (VectorE product):  t=clamp(x,Xk,Xk1) [DVE]; q=|a|s^2 [ScE Square,
     weight folded]; r=(t-d)*q [DVE scalar_tensor_tensor, 1x];
     psum += (g/h)*t [scaled ident] + sign(a)*r [+/-I]
  C2 (bf16 product):     t [DVE]; q_b=|a|s^2 ->bf16 [ScE]; lin_b=s+b/a ->bf16
     [ScE Identity act]; r=lin_b*q_b [DVE tensor_tensor bf16, 2x];
     psum += (g/h)*t + sign(a)*r [+/-I bf16]
  A  (ScalarE clamps):   w=Relu(u-k), v~=|g|(1-s) [2x ScE Relu];
     q=(|a|/|g|)s^2 [ScE]; r=(v~-c)*q [DVE STT]; psum += -sign(g)*v~ - sign(a)*r

TensorE accumulates all knots into PSUM via identity matmuls (fp32r/bf16,
1 cyc/row); ScalarE Identity(+A0' bias) evicts.  Identity matrices are built
once on the otherwise-idle GPSIMD (affine_select diagonal fill).

Sharding: pure data parallel - x split into 8 contiguous shards of 262144,
one per NeuronCore; knots/coeffs fold into immediates.
"""

import os
import numpy as np

N_TOTAL = 2_097_152
N_CORES = 8
P = 128
SHARD = N_TOTAL // N_CORES          # 262144
W = SHARD // P                      # 2048 fp32 per partition
BANK = 512                          # psum bank width (fp32)
NB = W // BANK                      # 4 banks

NUM_KNOTS = 64
DEG = 3
NI = NUM_KNOTS - 1                  # 63 intervals / knots in the s-basis

LAMBDA_A = float(os.environ.get("KERNEL_LAMBDA_A", "0.0"))
LAMBDA_C2 = float(os.environ.get("KERNEL_LAMBDA_C2", "0.48"))
ALPHA_EPS = 1e-7

_CACHE: dict = {}


def _tables(knots: np.ndarray, coeffs: np.ndarray):
    kd = knots.astype(np.float64)
    cd = coeffs.astype(np.float64)
    K = NUM_KNOTS - 1 - DEG          # 60 basis functions
    h = (kd[-1] - kd[0]) / (NUM_KNOTS - 1)
    assert np.allclose(np.diff(kd), h, rtol=1e-4, atol=1e-6), "knots not uniform"
    t0 = kd[0]

    def c(j):
        return cd[j] if 0 <= j < K else 0.0

    alp = np.zeros(NI)
    bet = np.zeros(NI)
    gam = np.zeros(NI)
    for m in range(NI):
        gam[m] = c(m - 2) / 3.0 + c(m - 1) / 3.0
        bet[m] = c(m - 1) / 6.0
        alp[m] = c(m) / 6.0
    a0 = (2.0 / 3.0) * c(-2) + (1.0 / 6.0) * c(-1)   # == 0
    return h, t0, a0, alp, bet, gam


def _plan(knots, coeffs):
    h, t0, a0, alp, bet, gam = _tables(knots, coeffs)
    su = 1.0 / h

    plan = []
    fills = []        # scaled f32r identity fills (linear channel of C/C2)
    const = a0
    n_a = n_c2 = 0
    for k in range(NI):
        a_, b_, g_ = alp[k], bet[k], gam[k]
        if a_ == 0.0 and b_ == 0.0 and g_ == 0.0:
            continue
        Xk = t0 + k * h
        Xk1 = t0 + (k + 1) * h
        cub = abs(a_) > ALPHA_EPS * max(abs(b_), 1.0)
        nplan = len(plan) + 1
        if (n_a + 1) / nplan <= LAMBDA_A:
            recipe = "A"
            n_a += 1
        elif cub and (n_c2 + 1) / nplan <= LAMBDA_C2:
            recipe = "C2"
            n_c2 += 1
        else:
            recipe = "C"
        ent = {"k": k, "recipe": recipe, "cubic": cub}
        if recipe == "A":
            # w = Relu(su*x + bu-k);  vt = G*(1-s), G=|g| (or 1 if g==0)
            G = abs(g_) if g_ != 0.0 else 1.0
            ent["w_bias"] = -t0 / h - k
            ent["v_scale"] = -G
            ent["v_bias"] = G
            if g_ != 0.0:
                ent["lin_sign"] = -np.sign(g_)       # psum += -g*v
                const += g_
            if cub:
                F = abs(a_) / G                       # q = F*G^2... -> |a| s^2/G
                ent["q_scale"] = -np.sqrt(F) / G
                ent["q_bias"] = np.sqrt(F)
                ent["stt_c"] = G * (a_ + b_) / a_
                ent["cub_sign"] = -np.sign(a_)
            elif b_ != 0.0:
                Fb = abs(b_)
                ent["q_scale"] = -np.sqrt(Fb) / G
                ent["q_bias"] = np.sqrt(Fb)
                ent["sq_sign"] = np.sign(b_)
        else:
            ent["clamp"] = (Xk, Xk1)
            if g_ != 0.0:
                ent["w_lin"] = len(fills)
                fills.append(g_ / h)
                const += -g_ * Xk / h
            if cub:
                d = Xk - h * b_ / a_
                if recipe == "C2":
                    # q_b = |a| s^2 (bf16); lin_b = su*(t-d) (bf16)
                    sc = np.sqrt(abs(a_)) * su
                    ent["q_scale"] = sc
                    ent["q_bias"] = -sc * Xk
                    ent["lin_scale"] = su
                    ent["lin_bias"] = -su * d
                    ent["cub_sign"] = np.sign(a_)
                else:
                    # q = (|a|/h) s^2 ; r = (t-d)*q ; weight sign(a)
                    sc = np.sqrt(abs(a_) / h) * su
                    ent["q_scale"] = sc
                    ent["q_bias"] = -sc * Xk
                    ent["stt_c"] = d
                    ent["cub_sign"] = np.sign(a_)
            elif b_ != 0.0:
                sc = np.sqrt(abs(b_)) * su
                ent["q_scale"] = sc
                ent["q_bias"] = -sc * Xk
                ent["sq_sign"] = np.sign(b_)
        plan.append(ent)
    return plan, fills, const, su, t0, h


def _build(knots: np.ndarray, coeffs: np.ndarray):
    from concourse import bacc, mybir
    import concourse.tile as tile

    plan, fills, const, su, t0, h = _plan(knots, coeffs)
    nwl = len(fills)

    nc = bacc.Bacc("TRN2", target_bir_lowering=False, debug=False,
                   num_devices=N_CORES)
    f32 = mybir.dt.float32
    f32r = mybir.dt.float32r
    bf16 = mybir.dt.bfloat16
    Alu = mybir.AluOpType
    Act = mybir.ActivationFunctionType

    x_dram = nc.dram_tensor("x", [SHARD], f32, kind="ExternalInput")
    out_dram = nc.dram_tensor("out", [SHARD], f32, kind="ExternalOutput")
    x_2d = x_dram.ap().rearrange("(p w) -> p w", p=P)
    out_2d = out_dram.ap().rearrange("(p w) -> p w", p=P)

    # ScalarE activation biases must be [P,1] APs
    bias_vals: list[float] = []

    def bias_idx(v: float) -> int:
        v = float(np.float32(v))
        if v not in bias_vals:
            bias_vals.append(v)
        return bias_vals.index(v)

    for ent in plan:
        if "q_scale" in ent:
            ent["q_bias_i"] = bias_idx(ent["q_bias"])
        if ent["recipe"] == "A":
            ent["w_bias_i"] = bias_idx(ent["w_bias"])
            ent["v_bias_i"] = bias_idx(ent["v_bias"])
        if ent["recipe"] == "C2":
            ent["lin_bias_i"] = bias_idx(ent["lin_bias"])
    const_i = bias_idx(const)

    with tile.TileContext(nc) as tc:
        with (
            tc.tile_pool(name="const", bufs=1) as cpool,
            tc.tile_pool(name="work", bufs=3) as work,
            tc.tile_pool(name="outp", bufs=1) as outp,
            tc.tile_pool(name="psum", bufs=1, space="PSUM") as psum,
        ):
            biases = cpool.tile([P, max(len(bias_vals), 1)], f32, tag="biases")
            for bi, bv in enumerate(bias_vals):
                nc.gpsimd.memset(biases[:, bi:bi + 1], bv)

            zcol = cpool.tile([P, 1], f32, tag="zcol")
            nc.gpsimd.memset(zcol[:], 0.0)
            zbc = zcol[:].broadcast_to([P, P])

            def ident_fill(dst, val):
                nc.gpsimd.affine_select(
                    out=dst, in_=zbc, compare_op=Alu.not_equal,
                    fill=float(val), base=0, pattern=[[-1, P]],
                    channel_multiplier=1)

            # unscaled +/-1 identities in both matmul dtypes
            ipr = cpool.tile([P, P], f32r, tag="ipr")
            imr = cpool.tile([P, P], f32r, tag="imr")
            ipb = cpool.tile([P, P], bf16, tag="ipb")
            imb = cpool.tile([P, P], bf16, tag="imb")
            ident_fill(ipr[:], 1.0)
            ident_fill(imr[:], -1.0)
            ident_fill(ipb[:], 1.0)
            ident_fill(imb[:], -1.0)

            def sgn_r(s):
                return ipr if s > 0 else imr

            def sgn_b(s):
                return ipb if s > 0 else imb

            # scaled identities (linear channel of C/C2), built in knot order
            identl = cpool.tile([P, max(nwl, 1) * P], f32r, tag="identl")
            for wi, val in enumerate(fills):
                ident_fill(identl[:, wi * P:(wi + 1) * P], val)

            x_tile = cpool.tile([P, W], f32, tag="x")
            for b in range(NB):
                nc.sync.dma_start(out=x_tile[:, b * BANK:(b + 1) * BANK],
                                  in_=x_2d[:, b * BANK:(b + 1) * BANK])

            acc = [psum.tile([P, BANK], f32, tag=f"ps{b}", name=f"ps{b}")
                   for b in range(NB)]
            started = [False] * NB
            n_mm_total = 0
            for e in plan:
                n_mm_total += ("w_lin" in e or "lin_sign" in e)
                n_mm_total += ("cub_sign" in e or "sq_sign" in e)
            mm_done = [0] * NB

            def mm(b, lhsT, rhs):
                first = not started[b]
                started[b] = True
                mm_done[b] += 1
                nc.tensor.matmul(
                    acc[b][:], lhsT,
                    rhs[:, b * BANK:(b + 1) * BANK],
                    start=first, stop=(mm_done[b] == n_mm_total))

            def bias_ap(i):
                return biases[:, i:i + 1]

            for ent in plan:
                k = ent["k"]
                rec = ent["recipe"]
                if rec == "A":
                    wt = work.tile([P, W], f32, tag="w", name=f"w{k}")
                    nc.scalar.activation(wt[:], x_tile[:], Act.Relu,
                                         bias=bias_ap(ent["w_bias_i"]),
                                         scale=float(su))
                    vt = work.tile([P, W], f32r, tag="v", name=f"v{k}")
                    nc.scalar.activation(vt[:], wt[:], Act.Relu,
                                         bias=bias_ap(ent["v_bias_i"]),
                                         scale=float(ent["v_scale"]))
                    if "lin_sign" in ent:
                        for b in range(NB):
                            mm(b, sgn_r(ent["lin_sign"])[:], vt)
                    if "q_scale" in ent:
                        qt = work.tile([P, W], f32r, tag="q", name=f"q{k}")
                        nc.scalar.activation(qt[:], vt[:], Act.Square,
                                             bias=bias_ap(ent["q_bias_i"]),
                                             scale=float(ent["q_scale"]))
                        if "stt_c" in ent:
                            rt = work.tile([P, W], f32r, tag="r", name=f"r{k}")
                            nc.vector.scalar_tensor_tensor(
                                rt[:], vt[:], float(ent["stt_c"]), qt[:],
                                Alu.subtract, Alu.mult)
                            for b in range(NB):
                                mm(b, sgn_r(ent["cub_sign"])[:], rt)
                        else:
                            for b in range(NB):
                                mm(b, sgn_r(ent["sq_sign"])[:], qt)
                else:
                    Xk, Xk1 = ent["clamp"]
                    tt = work.tile([P, W], f32r, tag="t", name=f"t{k}")
                    nc.vector.tensor_scalar(tt[:], x_tile[:],
                                            float(Xk), float(Xk1),
                                            Alu.max, Alu.min)
                    if "w_lin" in ent:
                        wi = ent["w_lin"]
                        for b in range(NB):
                            mm(b, identl[:, wi * P:(wi + 1) * P], tt)
                    if "q_scale" in ent:
                        qdt = bf16 if rec == "C2" else f32r
                        qt = work.tile([P, W], qdt, tag=f"q{qdt.name}",
                                       name=f"q{k}")
                        nc.scalar.activation(qt[:], tt[:], Act.Square,
                                             bias=bias_ap(ent["q_bias_i"]),
                                             scale=float(ent["q_scale"]))
                        if rec == "C2":
                            lb = work.tile([P, W], bf16, tag="lb", name=f"l{k}")
                            nc.scalar.activation(lb[:], tt[:], Act.Identity,
                                                 bias=bias_ap(ent["lin_bias_i"]),
                                                 scale=float(ent["lin_scale"]))
                            rt = work.tile([P, W], bf16, tag="rb", name=f"r{k}")
                            nc.vector.tensor_tensor(rt[:], lb[:], qt[:], Alu.mult)
                            for b in range(NB):
                                mm(b, sgn_b(ent["cub_sign"])[:], rt)
                        elif "stt_c" in ent:
                            rt = work.tile([P, W], f32r, tag="r", name=f"r{k}")
                            nc.vector.scalar_tensor_tensor(
                                rt[:], tt[:], float(ent["stt_c"]), qt[:],
                                Alu.subtract, Alu.mult)
                            for b in range(NB):
                                mm(b, sgn_r(ent["cub_sign"])[:], rt)
                        else:
                            for b in range(NB):
                                mm(b, sgn_r(ent["sq_sign"])[:], qt)

            out_tile = outp.tile([P, W], f32, tag="o")
            for b in range(NB):
                nc.scalar.activation(
                    out_tile[:, b * BANK:(b + 1) * BANK], acc[b][:],
                    Act.Identity, bias=bias_ap(const_i), scale=1.0)
            nc.sync.dma_start(out=out_2d, in_=out_tile[:])

    nc.compile()
    return nc


def _get_nc(knots: np.ndarray, coeffs: np.ndarray):
    key = (knots.astype(np.float32).tobytes(),
           coeffs.astype(np.float32).tobytes(), LAMBDA_A, LAMBDA_C2)
    if key not in _CACHE:
        _CACHE[key] = _build(knots, coeffs)
    return _CACHE[key]


LAST_RESULT = None


def _ensure_trace_hook() -> bool:
    """The image's antenv lacks axon_hooks; shim it so trace=True works."""
    try:
        from antenv.axon_hooks import get_axon_ntff_profile_hook  # noqa: F401
        return True
    except ImportError:
        pass
    try:
        import sys
        import types
        mod = types.ModuleType("antenv.axon_hooks")
        mod._hook = None

        def set_axon_ntff_profile_hook(hk):
            mod._hook = hk

        def get_axon_ntff_profile_hook():
            return mod._hook

        mod.set_axon_ntff_profile_hook = set_axon_ntff_profile_hook
        mod.get_axon_ntff_profile_hook = get_axon_ntff_profile_hook
        sys.modules["antenv.axon_hooks"] = mod
        import antenv
        antenv.axon_hooks = mod
        from trn_agent_boot.trn_boot import _ntff_profile_via_ctypes
        hook = _ntff_profile_via_ctypes("/opt/axon/libaxon_pjrt.so")
        mod._hook = hook
        return hook is not None
    except Exception:
        return False


def kernel(x: np.ndarray, knots: np.ndarray, coeffs: np.ndarray) -> np.ndarray:
    global LAST_RESULT
    from concourse.bass_utils import run_bass_kernel_spmd

    x = np.ascontiguousarray(np.asarray(x, dtype=np.float32))
    assert x.shape == (N_TOTAL,)
    nc = _get_nc(np.asarray(knots), np.asarray(coeffs))

    shards = x.reshape(N_CORES, SHARD)
    in_maps = [{"x": np.ascontiguousarray(shards[i])} for i in range(N_CORES)]
    trace = bool(int(os.environ.get("KERNEL_TRACE", "0")))
    if trace:
        trace = _ensure_trace_hook()
    res = run_bass_kernel_spmd(
        nc, in_maps, core_ids=list(range(N_CORES)), trace=trace)
    LAST_RESULT = res
    out = np.concatenate([res.results[i]["out"].reshape(-1)
                          for i in range(N_CORES)])
    return out.astype(np.float32, copy=False)
